# revision 15
# baseline (speedup 1.0000x reference)
"""EventTrace kernel for Trainium2 (8 NeuronCores, Bass/Tile).

Computes, for each batch row b:
    ev[t]   = embed[ctrl_tokens[b, t, 1]]          (gather from [64,512] table)
    c[t]    = ALPHA * c[t-1] + ev[t],  c[-1] = prev_trace[b]
    out[b]  = c                                     -> [B, T, D] float32

Algorithm (per core, 2 batch rows):
  Instead of gathering 16 MiB of embeddings, scan *decayed one-hot counts*
  G[v, t] = ALPHA * G[v, t-1] + onehot(idx_t == v) on the vector engine
  (fp32 internal state, fp16 output; both rows in one [128, T] scan), then
  reconstruct each output group with K=64 fp16 matmuls per row:
      C[t, d] = sum_v G[v, t] * embed[v, d]
  The two rows' matmuls use PE row-tiling (tile_position (0,0) / (64,0)) so
  they run concurrently.

  Time is processed in 512-step groups with a STRIDE-4 interleave: matmul
  j (j = 0..3) of a group uses the strided weight slice G[:, g*512+j::4],
  so output partition p holds timesteps 4p+j.  After the four [128, 512]
  matmuls are evicted (PSUM f32 -> SBUF bf16, two [128, 1024] two-bank
  copies split between DVE and ACT), each SBUF partition holds FOUR
  consecutive DRAM t-rows = one contiguous 4 KiB bf16 line, so the output
  DMA uses large packets and 1/4 the descriptor count.

  The prev-trace carry (prev * ALPHA^(t+1), relevant only for t < ~330)
  is added in PSUM by tiny K=1 accumulate-matmuls (alpha-powers x prev)
  in group 0, keeping DVE/ACT free of extra work.

Sharding: batch rows across the 8 cores (2 rows per core); the embedding
table and constants are replicated.  Output is written bf16 and upcast on
host (rel-err ~2e-3, well within tolerance).
"""

import sys

for _p in ("/root/.axon_site/_ro/trn_rl_repo", "/opt/trn_rl_repo"):
    if _p not in sys.path:
        sys.path.append(_p)

import numpy as np

import concourse.bass as bass
import concourse.tile as tile
from concourse import mybir
from concourse.bass_utils import run_bass_kernel_spmd

ALPHA = 0.9
B, T, V, D = 16, 4096, 64, 512
NCORES = 8
RPC = B // NCORES  # batch rows per core
GRP = 512  # timesteps per output group (stride-4 interleave)
NGRP = T // GRP
# scan/pipeline chunk boundaries (in timesteps); first chunk small so the
# matmul pipeline starts early.  Each chunk holds whole groups.
CHUNKS = [512, 512, 1024, 1024, 1024]
assert sum(CHUNKS) == T and all(c % GRP == 0 for c in CHUNKS)

F32 = mybir.dt.float32
F16 = mybir.dt.float16
BF16 = mybir.dt.bfloat16

# group-row index -> eviction engine for that group-row's single 4-bank
# PSUM->SBUF copy.  PSUM reads run at 1 elem/cycle/partition on both
# engines (no 2x modes), so ACT (1.2 GHz, no other work) takes the larger
# share; DVE also runs is_equal+scan+cast.  {0, 2, 6, 10} MUST be DVE:
# their PSUM slots are reused by the first matmul group of the next chunk,
# whose WAR wait must ride the DVE stream (the scan wait there is implied
# by PE program order only within a chunk).
_DVE_ROWS = frozenset({0, 2, 6, 10})


def _evict_engine(i):
    return "dve" if i in _DVE_ROWS else "act"


# f16in column layout: embedding rhs | prev rhs | alpha-power weights
FI_E, FI_PREV, FI_APOW = 0, D, 2 * D
FI_W = 3 * D


def build_nc(strip=True):
    nc = bass.Bass(trn_type="TRN2", target_bir_lowering=False)

    # idx[b] broadcast across partitions b*64..(b+1)*64, bf16 (values 0..63)
    idx_d = nc.dram_tensor("idxin", [128, T], BF16, kind="ExternalInput")
    # bf16 payload: embed duplicated into both halves, prev_trace[b] broadcast
    # per row-half, and alpha^(t+1) (t=0..511) on every partition.
    f16in_d = nc.dram_tensor("f16in", [128, FI_W], BF16, kind="ExternalInput")
    # tiny f32 header: col 0 = iota (0..63 twice), col 1 = ALPHA
    hdr_d = nc.dram_tensor("hdrin", [128, 2], F32, kind="ExternalInput")
    out = nc.dram_tensor("out", [RPC, T, D], BF16, kind="ExternalOutput")

    with tile.TileContext(nc) as tc:
        with (
            tc.tile_pool(name="const", bufs=1) as cpool,
            tc.tile_pool(name="psum", bufs=2, space="PSUM") as ppool,
            tc.tile_pool(name="outp", bufs=6) as opool,
        ):
            # latency-critical inputs ride HWDGE (fast); bulk idx chunks ride
            # SWDGE so they stay off the HW-DMA stream the output needs
            idx_t = cpool.tile([128, T], BF16, name="idx_t")
            f16in_t = cpool.tile([128, FI_W], BF16, name="f16in_t")
            hdr_t = cpool.tile([128, 2], F32, name="hdr_t")
            nc.sync.dma_start(hdr_t[:], hdr_d[:])
            nc.sync.dma_start(f16in_t[:], f16in_d[:])
            cs_list = [sum(CHUNKS[:i]) for i in range(len(CHUNKS) + 1)]
            nc.sync.dma_start(idx_t[:, 0 : CHUNKS[0]], idx_d[:, 0 : CHUNKS[0]])
            for c in range(1, len(CHUNKS)):
                nc.gpsimd.dma_start(
                    idx_t[:, cs_list[c] : cs_list[c + 1]],
                    idx_d[:, cs_list[c] : cs_list[c + 1]],
                )

            scr = cpool.tile([128, 8], F32, name="scr")
            nc.vector.memset(scr[:], 0.0)
            # DVE header observation: a tiny touch absorbs the hdr DMA wait
            # so is_equal/scan on DVE carry a single wait each.
            nc.vector.tensor_copy(scr[0:1, 1:2], hdr_t[0:1, 0:1])

            m2 = cpool.tile([128, T], BF16, name="m2")
            g2 = cpool.tile([128, T], F32, name="g2")
            g2b = cpool.tile([128, T], BF16, name="g2b")
            # PE HAM warm-up fodder: a small bf16 tile with no DMA
            # dependency, so the PE can start issuing matmuls ~3us in and
            # reach the warm (2.4 GHz) clock before the real matmuls start.
            wtile = cpool.tile([128, 128], BF16, name="wtile")
            nc.gpsimd.memset(wtile[:], 0.0)

            def scan_chunk(c):
                cs, ce = cs_list[c], cs_list[c + 1]
                # M[p, t] = 1.0 if idx[p//64, t] == (p % 64) else 0.0
                nc.vector.tensor_scalar(
                    m2[:, cs:ce],
                    idx_t[:, cs:ce],
                    hdr_t[:, 0:1],
                    None,
                    mybir.AluOpType.is_equal,
                )
                # G[p, t] = ALPHA * G[p, t-1] + M[p, t]   (both rows at once)
                # f32 in/out (the only fast TensorTensorScan uop), then a 2x
                # DVE cast to bf16 for the matmul weights.
                nc.vector.tensor_tensor_scan(
                    g2[:, cs:ce],
                    hdr_t[:, 1:2].broadcast_to((128, ce - cs)),
                    m2[:, cs:ce],
                    0.0 if c == 0 else g2[:, cs - 1 : cs],
                    mybir.AluOpType.mult,
                    mybir.AluOpType.add,
                )
                nc.vector.tensor_copy(g2b[:, cs:ce], g2[:, cs:ce])

            last_ots = []
            scan_chunk(0)
            # PE warm-up: ~3us of back-to-back small matmuls on wtile so the
            # HAM clock gate opens (1.2 -> 2.4 GHz) before the real matmuls.
            # The last one reads f16in, absorbing its DMA wait into the PE
            # stream so every real matmul carries a single (scan/WAR) wait.
            ps_warm = ppool.tile([128, 4 * D], F32, name="ps")
            for w in range(14):
                nc.tensor.matmul(
                    ps_warm[:, 0:128],
                    wtile[0:64, :],
                    wtile[0:64, :],
                    start=True,
                    stop=True,
                    tile_position=(0, 0),
                )
            nc.tensor.matmul(
                ps_warm[0:1, 0:1],
                f16in_t[0:1, 0:1],
                f16in_t[0:1, 0:1],
                start=True,
                stop=True,
                tile_position=(0, 0),
            )
            gr_i = 0  # group-row counter (for eviction-engine pattern)
            for c in range(len(CHUNKS)):
                if c + 1 < len(CHUNKS):
                    scan_chunk(c + 1)
                for g in range(cs_list[c] // GRP, cs_list[c + 1] // GRP):
                    for b in range(RPC):
                        wr = _evict_engine(gr_i)
                        ot = opool.tile([128, 4 * D], BF16, name="ot")
                        # 4-byte touch absorbs the WAR wait on this slot's
                        # prior out-DMA, so evictions wait only on their MM.
                        if wr == "act":
                            nc.scalar.copy(ot[0:1, 0:1], scr[0:1, 0:1])
                        else:
                            nc.vector.tensor_copy(ot[0:1, 0:1], scr[0:1, 0:1])
                        # one 4-bank PSUM tile per group-row (j = 0..3)
                        ps = ps_warm if gr_i == 0 else ppool.tile(
                            [128, 4 * D], F32, name="ps"
                        )
                        gr_i += 1
                        for j in range(4):
                            quarter = ps[:, j * D : (j + 1) * D]
                            nc.tensor.matmul(
                                quarter,
                                g2b[
                                    b * V : (b + 1) * V,
                                    g * GRP + j : (g + 1) * GRP : 4,
                                ],
                                f16in_t[b * V : (b + 1) * V, FI_E : FI_E + D],
                                start=True,
                                stop=(g > 0),
                                tile_position=(b * V, 0),
                            )
                            if g == 0:
                                # += alpha^(4p+j+1) * prev_b  (K=1 matmul)
                                nc.tensor.matmul(
                                    quarter,
                                    f16in_t[
                                        b * V : b * V + 1,
                                        FI_APOW + j : FI_APOW + GRP : 4,
                                    ],
                                    f16in_t[
                                        b * V : b * V + 1, FI_PREV : FI_PREV + D
                                    ],
                                    start=False,
                                    stop=True,
                                    tile_position=(b * V, 0),
                                )
                        # single 4-bank eviction (PSUM f32 -> SBUF bf16)
                        if wr == "act":
                            nc.scalar.copy(ot[:], ps[:])
                        else:
                            nc.vector.tensor_copy(ot[:], ps[:])
                        # one DMA per group-row: partition p holds timesteps
                        # g*512 + 4p + j (j=0..3) -> 4 KiB contiguous line
                        dview = out[b, g * GRP : (g + 1) * GRP, :].rearrange(
                            "(p four) d -> p four d", four=4
                        )
                        sview = ot[:].rearrange("p (four d) -> p four d", four=4)
                        nc.sync.dma_start(dview, sview)
                        last_ots.append(ot)
                        last_ots = last_ots[-16:]
            # End-of-kernel sinks: writing each of the last 16 output slots
            # makes the DVE stream transitively observe every DMA queue's
            # final completion, so the tail drain needs only one wait after
            # the redundant-wait strip below.
            for ot in last_ots:
                nc.vector.tensor_copy(ot[0:1, 0:1], scr[0:1, 0:1])
    if strip:
        _strip_redundant_waits(nc)
    return nc


def _strip_redundant_waits(nc):
    """Remove statically-implied semaphore waits (vector-clock analysis).

    The TRN2 instruction encodings here accept only ONE sync-wait command
    per instruction, but Tile emits extra waits for pool-slot reuse and the
    kernel-tail drain.  Many of those waits are statically implied by
    program order: engine queues execute in order, each DMA queue completes
    FIFO, and observing a semaphore value inherits every guarantee its
    updaters had.  This pass computes, for every instruction, the semaphore
    floor guaranteed at issue, and drops any wait already implied without
    it.  Straight-line (loop-free) programs only.
    """
    import concourse.mybir as mybir

    insts = []
    for fn in nc.m.functions:
        for bb in fn.blocks:
            for ins in bb.instructions:
                insts.append(ins)

    def waits(ins):
        si = ins.sync_info
        return list(si.on_wait) if si is not None else []

    def updates(ins):
        si = ins.sync_info
        return list(si.on_update) if si is not None else []

    # Streams: compute instructions execute in order per engine; a DMACopy's
    # *data completion* (its sem update) is FIFO per DMA queue, gated by its
    # trigger (engine stream) issue.
    def is_dma(ins):
        return type(ins).__name__ == "InstDMACopy"

    def dma_queue(ins):
        us = updates(ins)
        return us[0].ant_name if us else None

    # sem -> ordered list of (inst_index, add_value); single-updater-stream
    # sems only are used for transitive guarantees.
    sem_updaters = {}
    sem_streams = {}
    for i, ins in enumerate(insts):
        key = ("q", dma_queue(ins)) if is_dma(ins) else ("e", str(ins.engine))
        for u in updates(ins):
            if u.update_mode not in ("sem-inc", "sem-add-imm") or u.update_reg:
                sem_streams.setdefault(u.ant_name, set()).add("reg")
                continue
            sem_updaters.setdefault(u.ant_name, []).append((i, u.update_value))
            sem_streams.setdefault(u.ant_name, set()).add(key)

    single_stream_sems = {s for s, st in sem_streams.items() if len(st) == 1}

    # cumulative sem value right after instruction i's update
    cum_after = {}
    run = {}
    for i, ins in enumerate(insts):
        for u in updates(ins):
            if u.update_mode in ("sem-inc", "sem-add-imm") and not u.update_reg:
                run[u.ant_name] = run.get(u.ant_name, 0) + u.update_value
                cum_after[(i, u.ant_name)] = run[u.ant_name]

    prev_engine = {}
    prev_queue = {}
    last_e = {}
    last_q = {}
    for i, ins in enumerate(insts):
        ek = str(ins.engine)
        prev_engine[i] = last_e.get(ek)
        last_e[ek] = i
        if is_dma(ins):
            qk = dma_queue(ins)
            prev_queue[i] = last_q.get(qk)
            last_q[qk] = i

    n = len(insts)
    # disp[i]: sem floor guaranteed when instruction i dispatches (data-order
    # level).  done[i]: floor when its effects (sem updates) are visible —
    # for a DMACopy that is DATA completion on its queue.
    disp = [dict() for _ in range(n)]
    done = [dict() for _ in range(n)]

    def join_into(dst, src):
        changed = False
        for s, v in src.items():
            if dst.get(s, 0) < v:
                dst[s] = v
                changed = True
        return changed

    def guarantee_of_wait(sem, val):
        """Floor implied by observing sem >= val."""
        out = {sem: val}
        if sem not in single_stream_sems:
            return out
        cum = 0
        for j, add in sem_updaters.get(sem, []):
            cum += add
            join_into(out, done[j])
            if cum >= val:
                break
        return out

    def disp_floor(i, skip_wait=None):
        out = {}
        p = prev_engine[i]
        if p is not None:
            join_into(out, disp[p])
            if not is_dma(insts[p]):
                # same-engine execution is in-order: p's effects precede i's
                join_into(out, done[p])
        for w in waits(insts[i]):
            if w is skip_wait:
                continue
            if w.wait_mode == "sem-ge-imm" and not w.wait_reg:
                join_into(out, guarantee_of_wait(w.ant_name, w.wait_value))
        return out

    def recompute():
        changed = True
        while changed:
            changed = False
            for i, ins in enumerate(insts):
                f = disp_floor(i)
                if join_into(disp[i], f):
                    changed = True
                d = dict(disp[i])
                if is_dma(ins):
                    pq = prev_queue.get(i)
                    if pq is not None:
                        join_into(d, done[pq])
                for u in updates(ins):
                    c = cum_after.get((i, u.ant_name))
                    if c is not None and d.get(u.ant_name, 0) < c:
                        d[u.ant_name] = c
                if join_into(done[i], d):
                    changed = True

    recompute()
    # Iteratively remove implied waits (one at a time, recomputing floors).
    for _round in range(2000):
        victim = None
        for i, ins in enumerate(insts):
            ws = waits(ins)
            if len(ws) < 2:
                continue
            for w in ws:
                if w.wait_mode != "sem-ge-imm" or w.wait_reg:
                    continue
                # A DMA trigger's wait on its OWN queue's semaphore is ring
                # backpressure, not a data dependency: same-queue DMAs
                # complete FIFO regardless, and this kernel keeps well under
                # the HWDGE ring depth per queue.  Droppable.
                if is_dma(ins) and w.ant_name == dma_queue(ins):
                    victim = (i, w)
                    break
                f = disp_floor(i, skip_wait=w)
                if f.get(w.ant_name, 0) >= w.wait_value:
                    victim = (i, w)
                    break
            if victim:
                break
        if victim is None:
            break
        i, w = victim
        si = insts[i].sync_info
        kept = [x for x in si.on_wait if x is not w]
        insts[i].sync_info = mybir.SyncInfo(on_wait=kept, on_update=si.on_update)
        for d in disp:
            d.clear()
        for d in done:
            d.clear()
        recompute()

    bad = [
        (type(ins).__name__, [(w.ant_name, w.wait_value) for w in waits(ins)])
        for ins in insts
        if len(waits(ins)) >= 2
    ]
    if bad:
        raise RuntimeError(f"instructions still carry >=2 waits: {bad[:5]}")


def make_in_maps(ctrl_tokens, prev_trace, embed):
    import ml_dtypes

    bf16 = ml_dtypes.bfloat16
    idx = np.asarray(ctrl_tokens)[:, :, 1].astype(bf16)  # [B, T] (< 64)
    prev = np.asarray(prev_trace, dtype=np.float32).astype(bf16)  # [B, D]
    emb = np.asarray(embed, dtype=np.float32).astype(bf16)  # [V, D]
    iota = np.arange(V, dtype=np.float32)
    apow = (ALPHA ** (np.arange(GRP, dtype=np.float64) + 1.0)).astype(bf16)
    hdr = np.empty((128, 2), np.float32)
    hdr[:, 0] = np.concatenate([iota, iota])
    hdr[:, 1] = ALPHA
    in_maps = []
    for c in range(NCORES):
        rows = [RPC * c + r for r in range(RPC)]
        idxin = np.empty((128, T), bf16)
        f16in = np.empty((128, FI_W), bf16)
        for r, b in enumerate(rows):
            idxin[r * V : (r + 1) * V, :] = idx[b][None, :]
            f16in[r * V : (r + 1) * V, FI_E : FI_E + D] = emb
            f16in[r * V : (r + 1) * V, FI_PREV : FI_PREV + D] = prev[b][None, :]
        f16in[:, FI_APOW : FI_APOW + GRP] = apow[None, :]
        in_maps.append({"idxin": idxin, "f16in": f16in, "hdrin": hdr})
    return in_maps


_NC_CACHE = None


def get_nc():
    global _NC_CACHE
    if _NC_CACHE is None:
        _NC_CACHE = build_nc()
    return _NC_CACHE


def kernel(ctrl_tokens, prev_trace, embed):
    in_maps = make_in_maps(ctrl_tokens, prev_trace, embed)
    res = run_bass_kernel_spmd(get_nc(), in_maps, core_ids=list(range(NCORES)))
    out = np.concatenate(
        [np.asarray(r["out"]) for r in res.results], axis=0
    )  # [B, T, D] bf16
    return np.ascontiguousarray(out.astype(np.float32))
